# revision 1
# baseline (speedup 1.0000x reference)
"""DistMult+KBLN scoring kernel for 8 Trainium2 NeuronCores.

Math (eval mode, per reference):
    e1 = E[e1_idx]; r = R[r_idx]                       [B, D]
    score_l[b,e] = sum_d (e1*r)[b,d] * E[e,d]
    d~[b,e,l]    = (n_h[b,l] - num_lit[e,l] - c[l]) / sqrt(var[l])
    score_n[b,e] = sum_l nf[r_idx][b,l] * exp(-d~^2)
    out = sigmoid(score_l + score_n)                   [B, E]

Sharding: entity axis split row-wise across 8 cores (5000 entities each);
each core computes its [B, E/8] slice; host concatenates. No collectives.

Device algorithm per core:
  - phi tiles [L=100(part), F(free)] produced in ONE ScalarE pass per batch
    row using Derivative_Erf(x) = 2/sqrt(pi)*exp(-x^2) with per-partition
    bias = -a~[b,:]  (a~ = (n_h - c)/sqrt(var), m~ = num_lit/sqrt(var)).
  - weighted sum over l on TensorE (f16): lhsT[b] = [100, 64] block with
    column b holding nf[r[b]]*sqrt(pi)/2; 64 matmuls accumulate into PSUM
    on top of the score_l matmuls (lhsT = (e1*r)^T, rhs = E^T chunk).
    10 accumulators live as halves of 5 PSUM banks via matmul col-tiling.
  - final Sigmoid on ScalarE straight from PSUM, DMA out.
"""
import sys

if "/opt/trn_rl_repo" not in sys.path:
    sys.path.insert(0, "/opt/trn_rl_repo")

import numpy as np

import concourse.bass as bass
import concourse.mybir as mybir
import concourse.tile as _tile
from concourse import tile
from concourse.bass_utils import run_bass_kernel_spmd
from concourse.vector_clock import ScopedClock

B = 64
NUM_ENT = 40000
NUM_REL = 1345
DIM = 200
N_LIT = 100
NCORES = 8
ESH = NUM_ENT // NCORES  # 5000 entities per core

SUBW = 500     # matmul free dim (one PSUM bank holds 512 fp32)

f32 = mybir.dt.float32
f16 = mybir.dt.float16
AF = mybir.ActivationFunctionType


def _drain_and_barrier_split(self, tick_clock, wait_clock):
    # This walrus build rejects >1 sync-wait per instruction; the tail Drain
    # normally carries one wait per active processor. Collect them on a probe
    # NOP instead (split later by _split_multi_waits) and emit a clean drain.
    nc = self.nc
    probe = nc.sync.nop(nofuse=True, hint="tail_wait_probe")
    wait_clock.add_sem_waits(probe.ins, ScopedClock({None: tick_clock.global_clock}))
    nc.sync.drain()
    nc.all_engine_barrier()
    assert self.sems is not None
    popped = nc._tile_sem_poison_stack.pop()
    assert popped is self._sem_poison
    nc.clear_and_free_semaphores(list(self.sems.allocated().values()))
    nc.all_engine_barrier()


_tile.TileContext._drain_and_barrier = _drain_and_barrier_split


def _split_multi_waits(nc: bass.Bass) -> int:
    """Hoist all-but-one sync wait from every instruction onto standalone
    single-wait EventSemaphore instructions inserted just before it (same
    engine, same block). Needed because this walrus build errors with
    "Too many sync wait commands" on instructions carrying >1 wait."""
    n_split = 0
    for bb in nc.m.functions[0].blocks:
        new_insts = []
        for inst in bb.instructions:
            waits = list(inst.sync_info.on_wait) if inst.sync_info else []
            if len(waits) > 1:
                for sw in waits[:-1]:
                    ev = mybir.InstEventSemaphore(
                        name=nc.get_next_instruction_name(),
                        engine=inst.engine,
                        ins=[],
                        outs=[],
                        sync_info=mybir.SyncInfo(on_wait=[sw], on_update=[]),
                    )
                    nc.register_instruction(ev)
                    new_insts.append(ev)
                    n_split += 1
                inst.sync_info.on_wait = waits[-1:]
            new_insts.append(inst)
        bb.instructions[:] = new_insts
    return n_split


def build_nc() -> bass.Bass:
    nc = bass.Bass()

    mT_d = nc.dram_tensor("mT", [N_LIT, ESH], f32, kind="ExternalInput")
    ET_d = nc.dram_tensor("ET", [DIM, ESH], f16, kind="ExternalInput")
    xT_d = nc.dram_tensor("xT", [DIM, B], f16, kind="ExternalInput")
    nab_d = nc.dram_tensor("nab", [N_LIT, B], f32, kind="ExternalInput")
    wblk_d = nc.dram_tensor("wblk", [N_LIT, B * B], f16, kind="ExternalInput")
    out_d = nc.dram_tensor("out", [B, ESH], f32, kind="ExternalOutput")

    HALF = ESH // 2  # 2500
    NS = ESH // SUBW  # 10 sub-accumulators; 2 per PSUM bank (split partitions)

    with tile.TileContext(nc) as tc:
        with (
            tc.tile_pool(name="const", bufs=1) as cpool,
            tc.tile_pool(name="phi", bufs=3) as phipool,
            tc.tile_pool(name="ps", bufs=1, space=bass.MemorySpace.PSUM) as pspool,
            tc.tile_pool(name="acc", bufs=1) as accpool,
        ):
            nab_sb = cpool.tile([N_LIT, B], f32, tag="nab")
            mT_sb = cpool.tile([N_LIT, ESH], f32, tag="mT")
            wblk_sb = cpool.tile([N_LIT, B * B], f16, tag="wblk")
            xT0_sb = cpool.tile([128, B], f16, tag="xT0")
            xT1_sb = cpool.tile([DIM - 128, B], f16, tag="xT1")
            ET0_sb = cpool.tile([128, ESH], f16, tag="ET0")
            ET1_sb = cpool.tile([DIM - 128, ESH], f16, tag="ET1")
            out2 = accpool.tile([128, HALF], f32, tag="outsb")

            # First Gaussian pass needs nab + the first half of mT: split that
            # load across the three DGE paths so it lands as early as possible.
            nc.sync.dma_start(nab_sb[:], nab_d[:])
            nc.sync.dma_start(mT_sb[0:26, 0:HALF], mT_d[0:26, 0:HALF])
            nc.scalar.dma_start(mT_sb[26:58, 0:HALF], mT_d[26:58, 0:HALF])
            nc.gpsimd.dma_start(mT_sb[58:100, 0:HALF], mT_d[58:100, 0:HALF])
            nc.sync.dma_start(mT_sb[0:30, HALF:ESH], mT_d[0:30, HALF:ESH])
            nc.scalar.dma_start(mT_sb[30:100, HALF:ESH], mT_d[30:100, HALF:ESH])
            nc.sync.dma_start(wblk_sb[:], wblk_d[:])
            nc.gpsimd.dma_start(xT0_sb[:], xT_d[0:128, :])
            nc.gpsimd.dma_start(xT1_sb[:], xT_d[128:DIM, :])
            nc.sync.dma_start(ET0_sb[:], ET_d[0:128, :])
            nc.sync.dma_start(ET1_sb[:], ET_d[128:DIM, :])

            # 10 sub-accumulators in 5 PSUM banks: sub s -> bank s%5, and the
            # bank's partition half s//5 (matmul col-tiling, tile_position).
            ps = [
                pspool.tile([128, SUBW], f32, tag=f"ps{s}", name=f"ps_{s}")
                for s in range(NS // 2)
            ]

            def acc_mm(s, lhsT, rhs, start, stop):
                bank, half = s % 5, s // 5
                nc.tensor.matmul(
                    ps[bank][half * B : (half + 1) * B, :], lhsT, rhs,
                    start=start, stop=stop, tile_position=(0, half * B),
                )

            for b in range(B):
                phi = phipool.tile([N_LIT, ESH], f16, tag="phi")
                if b == 0:
                    # split so the first half starts before mT fully lands
                    for lo, hi in ((0, HALF), (HALF, ESH)):
                        nc.scalar.activation(
                            phi[:, lo:hi], mT_sb[:, lo:hi],
                            AF.Derivative_Erf, bias=nab_sb[:, b : b + 1],
                        )
                else:
                    nc.scalar.activation(
                        phi[:], mT_sb[:],
                        AF.Derivative_Erf, bias=nab_sb[:, b : b + 1],
                    )
                # on the last row, stop bank k (subs k and k+5) as early as
                # possible so the tail sigmoid chain starts sooner
                s_order = (0, 5, 1, 6, 2, 7, 3, 8, 4, 9) if b == B - 1 else range(NS)
                for s in s_order:
                    acc_mm(
                        s, wblk_sb[:, b * B : (b + 1) * B],
                        phi[:, s * SUBW : (s + 1) * SUBW],
                        start=(b == 0), stop=(b == B - 1),
                    )
                if 1 <= b <= NS:
                    # score_l: (e1*r) @ E^T, one sub-range per b-iteration so
                    # the extra PE work stays under the per-iteration slack
                    # (and never waits on the E^T load, off the tail path).
                    s = b - 1
                    c0 = s * SUBW
                    acc_mm(s, xT0_sb[:], ET0_sb[:, c0 : c0 + SUBW],
                           start=False, stop=False)
                    acc_mm(s, xT1_sb[:], ET1_sb[:, c0 : c0 + SUBW],
                           start=False, stop=False)

            # final sigmoid straight from PSUM, one full-width block per bank
            # (both partition-halves of a bank stop at adjacent matmuls)
            for bank in range(NS // 2):
                nc.scalar.activation(
                    out2[:, bank * SUBW : (bank + 1) * SUBW],
                    ps[bank][:],
                    AF.Sigmoid,
                )
            # rows 0:64 hold entities [0, 2500), rows 64:128 hold [2500, 5000);
            # quarter-granularity stores start as soon as their sigmoids land
            QW = 2 * SUBW + SUBW // 2  # 1250
            nc.sync.dma_start(out_d[:, 0:QW], out2[0:B, 0:QW])
            nc.scalar.dma_start(out_d[:, HALF : HALF + QW], out2[B:128, 0:QW])
            nc.sync.dma_start(out_d[:, QW:HALF], out2[0:B, QW:HALF])
            nc.scalar.dma_start(out_d[:, HALF + QW : ESH], out2[B:128, QW:HALF])

    _split_multi_waits(nc)
    return nc


def make_in_maps(e1_idx, r_idx, E_weight, R_weight, num_lit, c, var, nf_weights):
    e1_idx = np.asarray(e1_idx).astype(np.int64)
    r_idx = np.asarray(r_idx).astype(np.int64)
    E_weight = np.asarray(E_weight, dtype=np.float32)
    R_weight = np.asarray(R_weight, dtype=np.float32)
    num_lit = np.asarray(num_lit, dtype=np.float32)
    c = np.asarray(c, dtype=np.float32)
    var = np.asarray(var, dtype=np.float32)
    nf_weights = np.asarray(nf_weights, dtype=np.float32)

    f64 = np.float64
    g = 1.0 / np.sqrt(var.astype(f64))                       # [L]
    x = E_weight[e1_idx].astype(f64) * R_weight[r_idx].astype(f64)   # [B, D]
    xT = np.ascontiguousarray(x.T).astype(np.float16)  # [D, B]
    a = (num_lit[e1_idx].astype(f64) - c.astype(f64)) * g    # [B, L]
    nab = np.ascontiguousarray(-a.T).astype(np.float32)      # [L, B]
    W = nf_weights[r_idx].astype(f64) * (np.sqrt(np.pi) / 2.0)  # [B, L]
    wblk = np.zeros((N_LIT, B, B), dtype=np.float16)
    for b in range(B):
        wblk[:, b, b] = W[b].astype(np.float16)
    wblk = wblk.reshape(N_LIT, B * B)
    mT = (num_lit.astype(f64) * g).T.astype(np.float32)      # [L, E]
    ET = E_weight.T.astype(np.float16)               # [D, E]

    in_maps = []
    for core in range(NCORES):
        sl = slice(core * ESH, (core + 1) * ESH)
        in_maps.append(
            {
                "mT": np.ascontiguousarray(mT[:, sl]),
                "ET": np.ascontiguousarray(ET[:, sl]),
                "xT": xT,
                "nab": nab,
                "wblk": wblk,
            }
        )
    return in_maps


_NC_CACHE = []


def kernel(**inputs) -> np.ndarray:
    if not _NC_CACHE:
        _NC_CACHE.append(build_nc())
    nc = _NC_CACHE[0]
    in_maps = make_in_maps(**inputs)
    res = run_bass_kernel_spmd(nc, in_maps, list(range(NCORES)))
    return np.concatenate([res.results[i]["out"] for i in range(NCORES)], axis=1)

